# revision 10
# baseline (speedup 1.0000x reference)
"""MinkowskiSwitchNorm Trainium2 kernel (8 NeuronCores, Bass/Tile).

Math: everything derives from two per-segment sums S1[b,c]=sum(x), S2[b,c]=sum(x^2)
plus host-computed counts.  Each core computes partial S1/S2 on its shard of N
via one-hot matmuls (segment-sum on the PE), a tiny [8,128] AllReduce combines
them, stats are finished on-chip into per-segment tables A=inv_std*w and
D=b-mean*A, and a second pass computes out = x*A[id] + D[id] via a K=8
gather-matmul plus vector FMA.
"""

import numpy as np
import ml_dtypes
from contextlib import ExitStack

import concourse.bass as bass
import concourse.tile as tile
from concourse import bacc, mybir
from concourse.bass_utils import run_bass_kernel_spmd

NCORES = 8
B = 8            # segments
C = 64           # channels
NTOT = 1_000_000
NL = NTOT // NCORES      # rows per core
P = 128
W = 1024                 # point-groups of 128 per core (padded)
NLP = P * W              # padded rows per core = 131072
WC = 64                  # groups per chunk
NCH = W // WC            # chunks = 16
EPS = 1e-5
F32 = mybir.dt.float32
BF16 = mybir.dt.bfloat16

_CACHE = {}


def _build():
    nc = bacc.Bacc("TRN2", target_bir_lowering=False, debug=False,
                   num_devices=NCORES)

    xbf_i = nc.dram_tensor("xbf", [P, W * C], BF16, kind="ExternalInput").ap()
    xf_i = nc.dram_tensor("xf", [P, W * C], F32, kind="ExternalInput").ap()
    oh1_i = nc.dram_tensor("oh1", [P, W * B], BF16, kind="ExternalInput").ap()
    oh2_i = nc.dram_tensor("oh2", [2 * B, W * P], BF16, kind="ExternalInput").ap()
    w_i = nc.dram_tensor("wt", [1, C], F32, kind="ExternalInput").ap()
    b_i = nc.dram_tensor("bs", [1, C], F32, kind="ExternalInput").ap()
    hs_i = nc.dram_tensor("hs", [B, 8], F32, kind="ExternalInput").ap()
    c82_i = nc.dram_tensor("c82", [B, 2], F32, kind="ExternalInput").ap()
    out_o = nc.dram_tensor("out", [P, W * C], F32, kind="ExternalOutput").ap()

    cc_in = nc.dram_tensor("cc_in", [B, 2 * C], F32)
    cc_out = nc.dram_tensor("cc_out", [B, 2 * C], F32, addr_space="Shared")
    adx_d = nc.dram_tensor("adx_d", [2 * B, 2 * C], BF16)

    with ExitStack() as ctx:
        tc = ctx.enter_context(tile.TileContext(nc))
        poolA = ctx.enter_context(tc.tile_pool(name="xint", bufs=2))
        poolB = ctx.enter_context(tc.tile_pool(name="oh1", bufs=2))
        poolC = ctx.enter_context(tc.tile_pool(name="xf", bufs=2))
        poolD = ctx.enter_context(tc.tile_pool(name="ohT", bufs=2))
        poolO = ctx.enter_context(tc.tile_pool(name="outc", bufs=2))
        poolT = ctx.enter_context(tc.tile_pool(name="tmp", bufs=3))
        singles = ctx.enter_context(tc.tile_pool(name="singles", bufs=1))
        psumA = ctx.enter_context(tc.tile_pool(name="psA", bufs=2, space="PSUM"))
        psumB = ctx.enter_context(tc.tile_pool(name="psB", bufs=3, space="PSUM"))
        psumS = ctx.enter_context(tc.tile_pool(name="psS", bufs=1, space="PSUM"))

        # ---------------- pass 1: per-segment partial sums ----------------
        acc_sb = singles.tile([B, 2 * C], F32)
        for ci in range(NCH):
            xint = poolA.tile([P, WC * 2 * C], BF16)
            xint_r = xint[:].rearrange("p (w k) -> p w k", k=2 * C)
            src = xbf_i[:, ci * WC * C:(ci + 1) * WC * C].rearrange(
                "p (w c) -> p w c", c=C)
            nc.sync.dma_start(out=xint_r[:, :, 0:C], in_=src)
            nc.scalar.activation(out=xint_r[:, :, C:2 * C],
                                 in_=xint_r[:, :, 0:C],
                                 func=mybir.ActivationFunctionType.Square,
                                 scale=1.0)
            oh1t = poolB.tile([P, WC * B], BF16)
            nc.sync.dma_start(out=oh1t[:],
                              in_=oh1_i[:, ci * WC * B:(ci + 1) * WC * B])
            oh1_r = oh1t[:].rearrange("p (w b) -> p w b", b=B)

            ps12 = psumA.tile([B, 2 * C], F32)
            for w in range(WC):
                nc.tensor.matmul(out=ps12[:], lhsT=oh1_r[:, w, :],
                                 rhs=xint_r[:, w, :],
                                 start=(w == 0), stop=(w == WC - 1))
            if ci == 0:
                nc.vector.tensor_copy(out=acc_sb[:], in_=ps12[:])
            else:
                nc.vector.tensor_tensor(out=acc_sb[:], in0=acc_sb[:],
                                        in1=ps12[:], op=mybir.AluOpType.add)

        # ---------------- all-reduce partials ----------------
        nc.sync.dma_start(out=cc_in[:], in_=acc_sb[:])
        nc.gpsimd.collective_compute(
            "AllReduce", mybir.AluOpType.add,
            replica_groups=[list(range(NCORES))],
            ins=[cc_in[:]], outs=[cc_out[:]])
        s12 = singles.tile([B, 2 * C], F32)
        nc.sync.dma_start(out=s12[:], in_=cc_out[:])

        # ---------------- stats -> A/D tables ----------------
        hs = singles.tile([B, 8], F32)
        nc.sync.dma_start(out=hs[:], in_=hs_i[:])
        c82 = singles.tile([B, 2], F32)
        nc.sync.dma_start(out=c82[:], in_=c82_i[:])
        w8 = singles.tile([B, C], F32)
        nc.sync.dma_start(out=w8[:], in_=w_i[:].to_broadcast([B, C]))
        b8 = singles.tile([B, C], F32)
        nc.sync.dma_start(out=b8[:], in_=b_i[:].to_broadcast([B, C]))

        S1g = s12[:, 0:C]
        S2g = s12[:, C:2 * C]
        invc = hs[:, 0:1]

        mean_in = singles.tile([B, C], F32)
        nc.vector.tensor_scalar(out=mean_in[:], in0=S1g, scalar1=invc,
                                scalar2=None, op0=mybir.AluOpType.mult)
        E2 = singles.tile([B, C], F32)
        nc.vector.tensor_scalar(out=E2[:], in0=S2g, scalar1=invc,
                                scalar2=None, op0=mybir.AluOpType.mult)
        var_in = singles.tile([B, C], F32)
        nc.vector.tensor_tensor(out=var_in[:], in0=mean_in[:], in1=mean_in[:],
                                op=mybir.AluOpType.mult)
        nc.vector.tensor_tensor(out=var_in[:], in0=E2[:], in1=var_in[:],
                                op=mybir.AluOpType.subtract)

        mean_ln = singles.tile([B, 1], F32)
        nc.vector.reduce_sum(out=mean_ln[:], in_=mean_in[:],
                             axis=mybir.AxisListType.X)
        nc.vector.tensor_scalar(out=mean_ln[:], in0=mean_ln[:],
                                scalar1=1.0 / C, scalar2=None,
                                op0=mybir.AluOpType.mult)
        E2_ln = singles.tile([B, 1], F32)
        nc.vector.reduce_sum(out=E2_ln[:], in_=E2[:],
                             axis=mybir.AxisListType.X)
        var_ln = singles.tile([B, 1], F32)
        # var_ln = E2_ln/C - mean_ln^2
        nc.vector.tensor_scalar(out=E2_ln[:], in0=E2_ln[:], scalar1=1.0 / C,
                                scalar2=None, op0=mybir.AluOpType.mult)
        nc.vector.tensor_tensor(out=var_ln[:], in0=mean_ln[:], in1=mean_ln[:],
                                op=mybir.AluOpType.mult)
        nc.vector.tensor_tensor(out=var_ln[:], in0=E2_ln[:], in1=var_ln[:],
                                op=mybir.AluOpType.subtract)

        # column sums over segments (two M=1 matmuls so results sit on part 0)
        ps_cs1 = psumS.tile([1, 2 * C], F32)
        nc.tensor.matmul(out=ps_cs1[:], lhsT=c82[:, 0:1], rhs=s12[:],
                         start=True, stop=True)
        ps_cs2 = psumS.tile([1, 2 * C], F32)
        nc.tensor.matmul(out=ps_cs2[:], lhsT=c82[:, 1:2], rhs=s12[:],
                         start=True, stop=True)
        cs1 = singles.tile([1, 2 * C], F32)
        nc.vector.tensor_copy(out=cs1[:], in_=ps_cs1[:])
        cs2 = singles.tile([1, 2 * C], F32)
        nc.vector.tensor_copy(out=cs2[:], in_=ps_cs2[:])
        # mean_bn = cs1[0, 0:C] ;  S2/(N-1) = cs2[0, C:2C]
        mvbn = singles.tile([1, 2 * C], F32)
        nc.vector.tensor_copy(out=mvbn[:, 0:C], in_=cs1[:, 0:C])
        mbn2 = singles.tile([1, C], F32)
        nc.vector.tensor_tensor(out=mbn2[:], in0=cs1[:, 0:C],
                                in1=cs1[:, 0:C], op=mybir.AluOpType.mult)
        nc.vector.tensor_scalar(out=mbn2[:], in0=mbn2[:],
                                scalar1=float(NTOT) / float(NTOT - 1),
                                scalar2=None, op0=mybir.AluOpType.mult)
        nc.vector.tensor_tensor(out=mvbn[:, C:2 * C], in0=cs2[:, C:2 * C],
                                in1=mbn2[:], op=mybir.AluOpType.subtract)

        # broadcast [1,128] -> [8,128] via K=1 matmul with ones
        ones18 = singles.tile([1, B], F32)
        nc.vector.memset(ones18[:], 1.0)
        ps_bc = psumS.tile([B, 2 * C], F32)
        nc.tensor.matmul(out=ps_bc[:], lhsT=ones18[:], rhs=mvbn[:],
                         start=True, stop=True)
        bc = singles.tile([B, 2 * C], F32)
        nc.vector.tensor_copy(out=bc[:], in_=ps_bc[:])

        # mean = mw0*mean_in + mw1*mean_ln + mw2*mean_bn
        mls = singles.tile([B, 1], F32)
        nc.vector.tensor_tensor(out=mls[:], in0=mean_ln[:], in1=hs[:, 2:3],
                                op=mybir.AluOpType.mult)
        mean = singles.tile([B, C], F32)
        nc.vector.tensor_scalar(out=mean[:], in0=mean_in[:],
                                scalar1=hs[:, 1:2], scalar2=mls[:],
                                op0=mybir.AluOpType.mult,
                                op1=mybir.AluOpType.add)
        t2 = singles.tile([B, C], F32)
        nc.vector.tensor_scalar(out=t2[:], in0=bc[:, 0:C], scalar1=hs[:, 3:4],
                                scalar2=None, op0=mybir.AluOpType.mult)
        nc.vector.tensor_tensor(out=mean[:], in0=mean[:], in1=t2[:],
                                op=mybir.AluOpType.add)

        # var = vw0*var_in + vw1*var_ln + vw2*var_bn
        vls = singles.tile([B, 1], F32)
        nc.vector.tensor_tensor(out=vls[:], in0=var_ln[:], in1=hs[:, 5:6],
                                op=mybir.AluOpType.mult)
        var = singles.tile([B, C], F32)
        nc.vector.tensor_scalar(out=var[:], in0=var_in[:],
                                scalar1=hs[:, 4:5], scalar2=vls[:],
                                op0=mybir.AluOpType.mult,
                                op1=mybir.AluOpType.add)
        nc.vector.tensor_scalar(out=t2[:], in0=bc[:, C:2 * C],
                                scalar1=hs[:, 6:7], scalar2=None,
                                op0=mybir.AluOpType.mult)
        nc.vector.tensor_tensor(out=var[:], in0=var[:], in1=t2[:],
                                op=mybir.AluOpType.add)

        # inv_std = 1/sqrt(var+eps);  A = inv_std*w ; D = b - mean*A
        istd = singles.tile([B, C], F32)
        nc.scalar.activation(out=istd[:], in_=var[:],
                             func=mybir.ActivationFunctionType.Sqrt,
                             bias=hs[:, 7:8], scale=1.0)
        nc.vector.reciprocal(out=istd[:], in_=istd[:])
        AD = singles.tile([B, 2 * C], F32)
        nc.vector.tensor_tensor(out=AD[:, 0:C], in0=istd[:], in1=w8[:],
                                op=mybir.AluOpType.mult)
        mA = singles.tile([B, C], F32)
        nc.vector.tensor_tensor(out=mA[:], in0=mean[:], in1=AD[:, 0:C],
                                op=mybir.AluOpType.mult)
        nc.vector.tensor_tensor(out=AD[:, C:2 * C], in0=b8[:], in1=mA[:],
                                op=mybir.AluOpType.subtract)

        # split AD into bf16 hi+lo and stack into [16, 2C] via DRAM bounce
        ADhi = singles.tile([B, 2 * C], BF16)
        nc.vector.tensor_copy(out=ADhi[:], in_=AD[:])
        ADhi32 = singles.tile([B, 2 * C], F32)
        nc.vector.tensor_copy(out=ADhi32[:], in_=ADhi[:])
        ADlo32 = singles.tile([B, 2 * C], F32)
        nc.vector.tensor_tensor(out=ADlo32[:], in0=AD[:], in1=ADhi32[:],
                                op=mybir.AluOpType.subtract)
        ADlo = singles.tile([B, 2 * C], BF16)
        nc.vector.tensor_copy(out=ADlo[:], in_=ADlo32[:])
        nc.sync.dma_start(out=adx_d[0:B, :], in_=ADhi[:])
        nc.sync.dma_start(out=adx_d[B:2 * B, :], in_=ADlo[:])
        ADx = singles.tile([2 * B, 2 * C], BF16)
        nc.sync.dma_start(out=ADx[:], in_=adx_d[:])

        # ---------------- pass 2: normalize ----------------
        for ci in range(NCH):
            xf = poolC.tile([P, WC * C], F32)
            nc.sync.dma_start(out=xf[:],
                              in_=xf_i[:, ci * WC * C:(ci + 1) * WC * C])
            xf_r = xf[:].rearrange("p (w c) -> p w c", c=C)
            ohTt = poolD.tile([2 * B, WC * P], BF16)
            nc.sync.dma_start(out=ohTt[:],
                              in_=oh2_i[:, ci * WC * P:(ci + 1) * WC * P])
            outc = poolO.tile([P, WC * C], F32)
            outc_r = outc[:].rearrange("p (w c) -> p w c", c=C)

            for sg in range(WC // 4):
                psg = psumB.tile([P, 4 * 2 * C], F32)
                for m in range(4):
                    w = sg * 4 + m
                    nc.tensor.matmul(
                        out=psg[:, m * 2 * C:(m + 1) * 2 * C],
                        lhsT=ohTt[:, w * P:(w + 1) * P],
                        rhs=ADx[:],
                        start=True, stop=True)
                psg_r = psg[:].rearrange("p (g k) -> p g k", k=2 * C)
                tmp = poolT.tile([P, 4 * C], F32)
                tmp_r = tmp[:].rearrange("p (g c) -> p g c", c=C)
                nc.vector.tensor_tensor(out=tmp_r,
                                        in0=xf_r[:, sg * 4:(sg + 1) * 4, :],
                                        in1=psg_r[:, :, 0:C],
                                        op=mybir.AluOpType.mult)
                nc.vector.tensor_tensor(out=outc_r[:, sg * 4:(sg + 1) * 4, :],
                                        in0=tmp_r,
                                        in1=psg_r[:, :, C:2 * C],
                                        op=mybir.AluOpType.add)
            nc.sync.dma_start(out=out_o[:, ci * WC * C:(ci + 1) * WC * C],
                              in_=outc[:])

    nc.compile()
    return nc


def _get_nc():
    if "nc" not in _CACHE:
        _CACHE["nc"] = _build()
    return _CACHE["nc"]


def _softmax32(v):
    v = np.asarray(v, np.float32)
    e = np.exp(v - v.max())
    return (e / e.sum()).astype(np.float32)


def _prep_inputs(x, batch_ids, weight, bias, mean_weight, var_weight):
    x = np.asarray(x, np.float32)
    ids = np.asarray(batch_ids, np.int32)

    counts = np.bincount(ids, minlength=B).astype(np.float64)
    counts_c = np.maximum(counts, 1.0)
    mw = _softmax32(mean_weight)
    vw = _softmax32(var_weight)

    hs = np.zeros((B, 8), np.float32)
    hs[:, 0] = (1.0 / counts_c).astype(np.float32)
    hs[:, 1] = mw[0]
    hs[:, 2] = mw[1]
    hs[:, 3] = mw[2]
    hs[:, 4] = vw[0]
    hs[:, 5] = vw[1]
    hs[:, 6] = vw[2]
    hs[:, 7] = EPS
    c82 = np.zeros((B, 2), np.float32)
    c82[:, 0] = 1.0 / NTOT
    c82[:, 1] = 1.0 / (NTOT - 1)
    wt = np.ascontiguousarray(np.asarray(weight, np.float32).reshape(1, C))
    bs = np.ascontiguousarray(np.asarray(bias, np.float32).reshape(1, C))
    ar8 = np.arange(B, dtype=np.int32)

    in_maps = []
    for i in range(NCORES):
        xp = np.zeros((NLP, C), np.float32)
        xp[:NL] = x[i * NL:(i + 1) * NL]
        idp = np.full((NLP,), B, np.int32)
        idp[:NL] = ids[i * NL:(i + 1) * NL]

        xB = np.ascontiguousarray(
            xp.reshape(W, P, C).transpose(1, 0, 2)).reshape(P, W * C)
        oh1 = (idp.reshape(W, P)[:, :, None] == ar8)
        oh1 = np.ascontiguousarray(
            oh1.transpose(1, 0, 2)).reshape(P, W * B).astype(ml_dtypes.bfloat16)
        ohb = (idp[None, :] == ar8[:, None]).astype(ml_dtypes.bfloat16)
        oh2 = np.ascontiguousarray(np.concatenate([ohb, ohb], axis=0))

        in_maps.append(dict(
            xbf=np.ascontiguousarray(xB.astype(ml_dtypes.bfloat16)),
            xf=xB, oh1=oh1, oh2=oh2,
            wt=wt, bs=bs, hs=hs, c82=c82))
    return in_maps


def _postprocess(res):
    outs = []
    for i in range(NCORES):
        o = np.asarray(res.results[i]["out"], np.float32)
        o = o.reshape(P, W, C).transpose(1, 0, 2).reshape(NLP, C)[:NL]
        outs.append(o)
    return np.concatenate(outs, axis=0)


def kernel(x, batch_ids, weight, bias, mean_weight, var_weight):
    nc = _get_nc()
    in_maps = _prep_inputs(x, batch_ids, weight, bias,
                           mean_weight, var_weight)
    res = run_bass_kernel_spmd(nc, in_maps, list(range(NCORES)))
    _CACHE["last_result"] = res
    return _postprocess(res)


# revision 15
# speedup vs baseline: 1.1212x; 1.1212x over previous
"""MinkowskiSwitchNorm Trainium2 kernel (8 NeuronCores, Bass/Tile).

Math: everything derives from two per-segment sums S1[b,c]=sum(x), S2[b,c]=sum(x^2)
plus host-computed counts.  Each core computes partial S1/S2 on its shard of N
via one-hot matmuls (segment-sum on the PE), a tiny [8,128] AllReduce combines
them, stats are finished on-chip into per-segment tables A=inv_std*w and
D=b-mean*A, and a second pass computes out = x*A[id] + D[id] via a K=8
gather-matmul plus vector FMA.
"""

import numpy as np
import ml_dtypes
from contextlib import ExitStack

import concourse.bass as bass
import concourse.tile as tile
from concourse import bacc, mybir
from concourse.bass_utils import run_bass_kernel_spmd

NCORES = 8
B = 8            # segments
C = 64           # channels
NTOT = 1_000_000
NL = NTOT // NCORES      # rows per core
P = 128
W = 1024                 # point-groups of 128 per core (padded)
NLP = P * W              # padded rows per core = 131072
WC = 64                  # groups per chunk
NCH = W // WC            # chunks = 16
EPS = 1e-5
F32 = mybir.dt.float32
BF16 = mybir.dt.bfloat16

_CACHE = {}


def _build():
    nc = bacc.Bacc("TRN2", target_bir_lowering=False, debug=False,
                   num_devices=NCORES)

    xbf_i = nc.dram_tensor("xbf", [P, W * C], BF16, kind="ExternalInput").ap()
    xf_i = nc.dram_tensor("xf", [P, W * C], F32, kind="ExternalInput").ap()
    oh1_i = nc.dram_tensor("oh1", [P, W * B], BF16, kind="ExternalInput").ap()
    oh2_i = nc.dram_tensor("oh2", [2 * B, W * P], BF16, kind="ExternalInput").ap()
    w_i = nc.dram_tensor("wt", [1, C], F32, kind="ExternalInput").ap()
    b_i = nc.dram_tensor("bs", [1, C], F32, kind="ExternalInput").ap()
    hs_i = nc.dram_tensor("hs", [B, 8], F32, kind="ExternalInput").ap()
    c82_i = nc.dram_tensor("c82", [B, 2], F32, kind="ExternalInput").ap()
    out_o = nc.dram_tensor("out", [P, W * C], F32, kind="ExternalOutput").ap()

    cc_in = nc.dram_tensor("cc_in", [B, 2 * C], F32)
    cc_out = nc.dram_tensor("cc_out", [B, 2 * C], F32, addr_space="Shared")
    adx_d = nc.dram_tensor("adx_d", [2 * B, 2 * C], BF16)

    with ExitStack() as ctx:
        tc = ctx.enter_context(tile.TileContext(nc))
        poolA = ctx.enter_context(tc.tile_pool(name="xa", bufs=2))
        poolS = ctx.enter_context(tc.tile_pool(name="xs", bufs=2))
        poolB = ctx.enter_context(tc.tile_pool(name="oh1", bufs=2))
        poolC = ctx.enter_context(tc.tile_pool(name="xf", bufs=2))
        poolD = ctx.enter_context(tc.tile_pool(name="ohT", bufs=2))
        poolO = ctx.enter_context(tc.tile_pool(name="outc", bufs=2))
        poolT = ctx.enter_context(tc.tile_pool(name="tmp", bufs=3))
        singles = ctx.enter_context(tc.tile_pool(name="singles", bufs=1))
        psumA = ctx.enter_context(tc.tile_pool(name="psA", bufs=2, space="PSUM"))
        psumB = ctx.enter_context(tc.tile_pool(name="psB", bufs=2, space="PSUM"))
        psumS = ctx.enter_context(tc.tile_pool(name="psS", bufs=1, space="PSUM"))

        # ---------------- pass 1: per-segment partial sums ----------------
        acc_sb = singles.tile([B, 2 * C], F32)
        for ci in range(NCH):
            xa = poolA.tile([P, WC * C], BF16)
            nc.sync.dma_start(out=xa[:],
                              in_=xbf_i[:, ci * WC * C:(ci + 1) * WC * C])
            xs = poolS.tile([P, WC * C], BF16)
            nc.scalar.activation(out=xs[:], in_=xa[:],
                                 func=mybir.ActivationFunctionType.Square,
                                 scale=1.0)
            oh1t = poolB.tile([P, WC * B], BF16)
            nc.gpsimd.dma_start(out=oh1t[:],
                                in_=oh1_i[:, ci * WC * B:(ci + 1) * WC * B])
            oh1_r = oh1t[:].rearrange("p (w b) -> p w b", b=B)
            xa_r = xa[:].rearrange("p (w c) -> p w c", c=C)
            xs_r = xs[:].rearrange("p (w c) -> p w c", c=C)

            ps12 = psumA.tile([B, 2 * C], F32)
            for w in range(WC):
                nc.tensor.matmul(out=ps12[:, 0:C], lhsT=oh1_r[:, w, :],
                                 rhs=xa_r[:, w, :],
                                 start=(w == 0), stop=(w == WC - 1))
                nc.tensor.matmul(out=ps12[:, C:2 * C], lhsT=oh1_r[:, w, :],
                                 rhs=xs_r[:, w, :],
                                 start=(w == 0), stop=(w == WC - 1))
            if ci == 0:
                nc.vector.tensor_copy(out=acc_sb[:], in_=ps12[:])
            else:
                nc.vector.tensor_tensor(out=acc_sb[:], in0=acc_sb[:],
                                        in1=ps12[:], op=mybir.AluOpType.add)

        # ---------------- all-reduce partials ----------------
        nc.sync.dma_start(out=cc_in[:], in_=acc_sb[:])
        nc.gpsimd.collective_compute(
            "AllReduce", mybir.AluOpType.add,
            replica_groups=[list(range(NCORES))],
            ins=[cc_in[:]], outs=[cc_out[:]])
        s12 = singles.tile([B, 2 * C], F32)
        nc.sync.dma_start(out=s12[:], in_=cc_out[:])

        # ---------------- stats -> A/D tables ----------------
        hs = singles.tile([B, 8], F32)
        nc.sync.dma_start(out=hs[:], in_=hs_i[:])
        c82 = singles.tile([B, 2], F32)
        nc.sync.dma_start(out=c82[:], in_=c82_i[:])
        w8 = singles.tile([B, C], F32)
        nc.sync.dma_start(out=w8[:], in_=w_i[:].to_broadcast([B, C]))
        b8 = singles.tile([B, C], F32)
        nc.sync.dma_start(out=b8[:], in_=b_i[:].to_broadcast([B, C]))

        S1g = s12[:, 0:C]
        S2g = s12[:, C:2 * C]
        invc = hs[:, 0:1]

        mean_in = singles.tile([B, C], F32)
        nc.vector.tensor_scalar(out=mean_in[:], in0=S1g, scalar1=invc,
                                scalar2=None, op0=mybir.AluOpType.mult)
        E2 = singles.tile([B, C], F32)
        nc.vector.tensor_scalar(out=E2[:], in0=S2g, scalar1=invc,
                                scalar2=None, op0=mybir.AluOpType.mult)
        var_in = singles.tile([B, C], F32)
        nc.vector.tensor_tensor(out=var_in[:], in0=mean_in[:], in1=mean_in[:],
                                op=mybir.AluOpType.mult)
        nc.vector.tensor_tensor(out=var_in[:], in0=E2[:], in1=var_in[:],
                                op=mybir.AluOpType.subtract)

        mean_ln = singles.tile([B, 1], F32)
        nc.vector.reduce_sum(out=mean_ln[:], in_=mean_in[:],
                             axis=mybir.AxisListType.X)
        nc.vector.tensor_scalar(out=mean_ln[:], in0=mean_ln[:],
                                scalar1=1.0 / C, scalar2=None,
                                op0=mybir.AluOpType.mult)
        E2_ln = singles.tile([B, 1], F32)
        nc.vector.reduce_sum(out=E2_ln[:], in_=E2[:],
                             axis=mybir.AxisListType.X)
        var_ln = singles.tile([B, 1], F32)
        # var_ln = E2_ln/C - mean_ln^2
        nc.vector.tensor_scalar(out=E2_ln[:], in0=E2_ln[:], scalar1=1.0 / C,
                                scalar2=None, op0=mybir.AluOpType.mult)
        nc.vector.tensor_tensor(out=var_ln[:], in0=mean_ln[:], in1=mean_ln[:],
                                op=mybir.AluOpType.mult)
        nc.vector.tensor_tensor(out=var_ln[:], in0=E2_ln[:], in1=var_ln[:],
                                op=mybir.AluOpType.subtract)

        # column sums over segments (two M=1 matmuls so results sit on part 0)
        ps_cs = psumS.tile([1, 4 * C], F32)
        nc.tensor.matmul(out=ps_cs[:, 0:2 * C], lhsT=c82[:, 0:1], rhs=s12[:],
                         start=True, stop=True)
        nc.tensor.matmul(out=ps_cs[:, 2 * C:4 * C], lhsT=c82[:, 1:2],
                         rhs=s12[:], start=True, stop=True)
        cs1 = singles.tile([1, 2 * C], F32)
        nc.vector.tensor_copy(out=cs1[:], in_=ps_cs[:, 0:2 * C])
        cs2 = singles.tile([1, 2 * C], F32)
        nc.vector.tensor_copy(out=cs2[:], in_=ps_cs[:, 2 * C:4 * C])
        # mean_bn = cs1[0, 0:C] ;  S2/(N-1) = cs2[0, C:2C]
        mvbn = singles.tile([1, 2 * C], F32)
        nc.vector.tensor_copy(out=mvbn[:, 0:C], in_=cs1[:, 0:C])
        mbn2 = singles.tile([1, C], F32)
        nc.vector.tensor_tensor(out=mbn2[:], in0=cs1[:, 0:C],
                                in1=cs1[:, 0:C], op=mybir.AluOpType.mult)
        nc.vector.tensor_scalar(out=mbn2[:], in0=mbn2[:],
                                scalar1=float(NTOT) / float(NTOT - 1),
                                scalar2=None, op0=mybir.AluOpType.mult)
        nc.vector.tensor_tensor(out=mvbn[:, C:2 * C], in0=cs2[:, C:2 * C],
                                in1=mbn2[:], op=mybir.AluOpType.subtract)

        # broadcast [1,128] -> [8,128] via K=1 matmul with ones
        ones18 = singles.tile([1, B], F32)
        nc.vector.memset(ones18[:], 1.0)
        ps_bc = psumS.tile([B, 2 * C], F32)
        nc.tensor.matmul(out=ps_bc[:], lhsT=ones18[:], rhs=mvbn[:],
                         start=True, stop=True)
        bc = singles.tile([B, 2 * C], F32)
        nc.vector.tensor_copy(out=bc[:], in_=ps_bc[:])

        # mean = mw0*mean_in + mw1*mean_ln + mw2*mean_bn
        mls = singles.tile([B, 1], F32)
        nc.vector.tensor_tensor(out=mls[:], in0=mean_ln[:], in1=hs[:, 2:3],
                                op=mybir.AluOpType.mult)
        mean = singles.tile([B, C], F32)
        nc.vector.tensor_scalar(out=mean[:], in0=mean_in[:],
                                scalar1=hs[:, 1:2], scalar2=mls[:],
                                op0=mybir.AluOpType.mult,
                                op1=mybir.AluOpType.add)
        t2 = singles.tile([B, C], F32)
        nc.vector.tensor_scalar(out=t2[:], in0=bc[:, 0:C], scalar1=hs[:, 3:4],
                                scalar2=None, op0=mybir.AluOpType.mult)
        nc.vector.tensor_tensor(out=mean[:], in0=mean[:], in1=t2[:],
                                op=mybir.AluOpType.add)

        # var = vw0*var_in + vw1*var_ln + vw2*var_bn
        vls = singles.tile([B, 1], F32)
        nc.vector.tensor_tensor(out=vls[:], in0=var_ln[:], in1=hs[:, 5:6],
                                op=mybir.AluOpType.mult)
        var = singles.tile([B, C], F32)
        nc.vector.tensor_scalar(out=var[:], in0=var_in[:],
                                scalar1=hs[:, 4:5], scalar2=vls[:],
                                op0=mybir.AluOpType.mult,
                                op1=mybir.AluOpType.add)
        nc.vector.tensor_scalar(out=t2[:], in0=bc[:, C:2 * C],
                                scalar1=hs[:, 6:7], scalar2=None,
                                op0=mybir.AluOpType.mult)
        nc.vector.tensor_tensor(out=var[:], in0=var[:], in1=t2[:],
                                op=mybir.AluOpType.add)

        # inv_std = 1/sqrt(var+eps);  A = inv_std*w ; D = b - mean*A
        istd = singles.tile([B, C], F32)
        nc.scalar.activation(out=istd[:], in_=var[:],
                             func=mybir.ActivationFunctionType.Sqrt,
                             bias=hs[:, 7:8], scale=1.0)
        nc.vector.reciprocal(out=istd[:], in_=istd[:])
        AD = singles.tile([B, 2 * C], F32)
        nc.vector.tensor_tensor(out=AD[:, 0:C], in0=istd[:], in1=w8[:],
                                op=mybir.AluOpType.mult)
        mA = singles.tile([B, C], F32)
        nc.vector.tensor_tensor(out=mA[:], in0=mean[:], in1=AD[:, 0:C],
                                op=mybir.AluOpType.mult)
        nc.vector.tensor_tensor(out=AD[:, C:2 * C], in0=b8[:], in1=mA[:],
                                op=mybir.AluOpType.subtract)

        # split AD into bf16 hi+lo and stack into [16, 2C] via DRAM bounce
        ADhi = singles.tile([B, 2 * C], BF16)
        nc.vector.tensor_copy(out=ADhi[:], in_=AD[:])
        ADhi32 = singles.tile([B, 2 * C], F32)
        nc.vector.tensor_copy(out=ADhi32[:], in_=ADhi[:])
        ADlo32 = singles.tile([B, 2 * C], F32)
        nc.vector.tensor_tensor(out=ADlo32[:], in0=AD[:], in1=ADhi32[:],
                                op=mybir.AluOpType.subtract)
        ADlo = singles.tile([B, 2 * C], BF16)
        nc.vector.tensor_copy(out=ADlo[:], in_=ADlo32[:])
        nc.sync.dma_start(out=adx_d[0:B, :], in_=ADhi[:])
        nc.sync.dma_start(out=adx_d[B:2 * B, :], in_=ADlo[:])
        ADx = singles.tile([2 * B, 2 * C], BF16)
        nc.sync.dma_start(out=ADx[:], in_=adx_d[:])

        # ---------------- pass 2: normalize ----------------
        SG = 8  # groups per PSUM super-group (2 banks)
        for ci in range(NCH):
            xf = poolC.tile([P, WC * C], F32)
            nc.sync.dma_start(out=xf[:],
                              in_=xf_i[:, ci * WC * C:(ci + 1) * WC * C])
            xf_r = xf[:].rearrange("p (w c) -> p w c", c=C)
            ohTt = poolD.tile([2 * B, WC * P], BF16)
            nc.gpsimd.dma_start(out=ohTt[:],
                                in_=oh2_i[:, ci * WC * P:(ci + 1) * WC * P])
            outc = poolO.tile([P, WC * C], F32)
            outc_r = outc[:].rearrange("p (w c) -> p w c", c=C)

            for sg in range(WC // SG):
                psg = psumB.tile([P, SG * 2 * C], F32)
                for m in range(SG):
                    w = sg * SG + m
                    nc.tensor.matmul(
                        out=psg[:, m * 2 * C:(m + 1) * 2 * C],
                        lhsT=ohTt[:, w * P:(w + 1) * P],
                        rhs=ADx[:],
                        start=True, stop=True)
                psg_r = psg[:].rearrange("p (g k) -> p g k", k=2 * C)
                tmp = poolT.tile([P, SG * C], F32)
                tmp_r = tmp[:].rearrange("p (g c) -> p g c", c=C)
                nc.vector.tensor_tensor(out=tmp_r,
                                        in0=xf_r[:, sg * SG:(sg + 1) * SG, :],
                                        in1=psg_r[:, :, 0:C],
                                        op=mybir.AluOpType.mult)
                nc.vector.tensor_tensor(out=outc_r[:, sg * SG:(sg + 1) * SG, :],
                                        in0=tmp_r,
                                        in1=psg_r[:, :, C:2 * C],
                                        op=mybir.AluOpType.add)
            nc.gpsimd.dma_start(out=out_o[:, ci * WC * C:(ci + 1) * WC * C],
                                in_=outc[:])

    nc.compile()
    return nc


def _get_nc():
    if "nc" not in _CACHE:
        _CACHE["nc"] = _build()
    return _CACHE["nc"]


def _softmax32(v):
    v = np.asarray(v, np.float32)
    e = np.exp(v - v.max())
    return (e / e.sum()).astype(np.float32)


def _prep_inputs(x, batch_ids, weight, bias, mean_weight, var_weight):
    x = np.asarray(x, np.float32)
    ids = np.asarray(batch_ids, np.int32)

    counts = np.bincount(ids, minlength=B).astype(np.float64)
    counts_c = np.maximum(counts, 1.0)
    mw = _softmax32(mean_weight)
    vw = _softmax32(var_weight)

    hs = np.zeros((B, 8), np.float32)
    hs[:, 0] = (1.0 / counts_c).astype(np.float32)
    hs[:, 1] = mw[0]
    hs[:, 2] = mw[1]
    hs[:, 3] = mw[2]
    hs[:, 4] = vw[0]
    hs[:, 5] = vw[1]
    hs[:, 6] = vw[2]
    hs[:, 7] = EPS
    c82 = np.zeros((B, 2), np.float32)
    c82[:, 0] = 1.0 / NTOT
    c82[:, 1] = 1.0 / (NTOT - 1)
    wt = np.ascontiguousarray(np.asarray(weight, np.float32).reshape(1, C))
    bs = np.ascontiguousarray(np.asarray(bias, np.float32).reshape(1, C))
    ar8 = np.arange(B, dtype=np.int32)

    in_maps = []
    for i in range(NCORES):
        xp = np.zeros((NLP, C), np.float32)
        xp[:NL] = x[i * NL:(i + 1) * NL]
        idp = np.full((NLP,), B, np.int32)
        idp[:NL] = ids[i * NL:(i + 1) * NL]

        xB = np.ascontiguousarray(
            xp.reshape(W, P, C).transpose(1, 0, 2)).reshape(P, W * C)
        oh1 = (idp.reshape(W, P)[:, :, None] == ar8)
        oh1 = np.ascontiguousarray(
            oh1.transpose(1, 0, 2)).reshape(P, W * B).astype(ml_dtypes.bfloat16)
        ohb = (idp[None, :] == ar8[:, None]).astype(ml_dtypes.bfloat16)
        oh2 = np.ascontiguousarray(np.concatenate([ohb, ohb], axis=0))

        in_maps.append(dict(
            xbf=np.ascontiguousarray(xB.astype(ml_dtypes.bfloat16)),
            xf=xB, oh1=oh1, oh2=oh2,
            wt=wt, bs=bs, hs=hs, c82=c82))
    return in_maps


def _postprocess(res):
    outs = []
    for i in range(NCORES):
        o = np.asarray(res.results[i]["out"], np.float32)
        o = o.reshape(P, W, C).transpose(1, 0, 2).reshape(NLP, C)[:NL]
        outs.append(o)
    return np.concatenate(outs, axis=0)


def kernel(x, batch_ids, weight, bias, mean_weight, var_weight):
    nc = _get_nc()
    in_maps = _prep_inputs(x, batch_ids, weight, bias,
                           mean_weight, var_weight)
    res = run_bass_kernel_spmd(nc, in_maps, list(range(NCORES)))
    _CACHE["last_result"] = res
    return _postprocess(res)


# revision 18
# speedup vs baseline: 1.2798x; 1.1415x over previous
"""MinkowskiSwitchNorm Trainium2 kernel (8 NeuronCores, Bass/Tile).

Strategy: host sorts points by segment id so that every 8192-point device
chunk contains a single segment.  Pass 1 computes per-chunk column sums of
[x | x^2] with a ones-vector matmul and scatters each chunk sum into a
per-segment PSUM accumulator (K=1 matmul with the chunk's one-hot selector).
A tiny [8,128] AllReduce combines partials across the 8 cores, on-chip stats
produce per-segment tables A=inv_std*w, D=b-mean*A, and pass 2 normalizes:
out = x*A[seg] + D[seg], where A/D are broadcast per chunk via one small
matmul.  The host scatters rows back to their original order.
"""

import numpy as np
import ml_dtypes
from contextlib import ExitStack

import concourse.bass as bass
import concourse.tile as tile
from concourse import bacc, mybir
from concourse.bass_utils import run_bass_kernel_spmd

NCORES = 8
B = 8            # segments
C = 64           # channels
NTOT = 1_000_000
P = 128
W = 1024                 # point-groups of 128 per core
NLP = P * W              # padded rows per core = 131072
WC = 64                  # groups per chunk
NCH = W // WC            # chunks per core = 16
CHP = P * WC             # points per chunk = 8192
TOTCH = NCORES * NCH     # 128 chunks globally
EPS = 1e-5
F32 = mybir.dt.float32
BF16 = mybir.dt.bfloat16

_CACHE = {}


def _build():
    nc = bacc.Bacc("TRN2", target_bir_lowering=False, debug=False,
                   num_devices=NCORES)

    xbf_i = nc.dram_tensor("xbf", [P, W * C], BF16, kind="ExternalInput").ap()
    xf_i = nc.dram_tensor("xf", [P, W * C], F32, kind="ExternalInput").ap()
    selr_i = nc.dram_tensor("selr", [1, NCH * B], F32,
                            kind="ExternalInput").ap()
    selb_i = nc.dram_tensor("selb", [B, NCH * P], F32,
                            kind="ExternalInput").ap()
    w_i = nc.dram_tensor("wt", [1, C], F32, kind="ExternalInput").ap()
    b_i = nc.dram_tensor("bs", [1, C], F32, kind="ExternalInput").ap()
    hs_i = nc.dram_tensor("hs", [B, 8], F32, kind="ExternalInput").ap()
    c82_i = nc.dram_tensor("c82", [B, 2], F32, kind="ExternalInput").ap()
    out_o = nc.dram_tensor("out", [P, W * C], F32, kind="ExternalOutput").ap()

    cc_in = nc.dram_tensor("cc_in", [B, 2 * C], F32)
    cc_out = nc.dram_tensor("cc_out", [B, 2 * C], F32, addr_space="Shared")

    with ExitStack() as ctx:
        tc = ctx.enter_context(tile.TileContext(nc))
        poolA = ctx.enter_context(tc.tile_pool(name="xa", bufs=2))
        poolS = ctx.enter_context(tc.tile_pool(name="xs", bufs=2))
        poolC = ctx.enter_context(tc.tile_pool(name="xf", bufs=4))
        poolO = ctx.enter_context(tc.tile_pool(name="outc", bufs=2))
        poolT = ctx.enter_context(tc.tile_pool(name="tmp", bufs=2))
        poolAD = ctx.enter_context(tc.tile_pool(name="ad128", bufs=2))
        poolCP = ctx.enter_context(tc.tile_pool(name="cp12", bufs=2))
        singles = ctx.enter_context(tc.tile_pool(name="singles", bufs=1))
        psumA = ctx.enter_context(tc.tile_pool(name="psA", bufs=2, space="PSUM"))
        psumB = ctx.enter_context(tc.tile_pool(name="psB", bufs=2, space="PSUM"))
        psumS = ctx.enter_context(tc.tile_pool(name="psS", bufs=1, space="PSUM"))

        ones128 = singles.tile([P, 1], BF16)
        nc.vector.memset(ones128[:], 1.0)
        selrow = singles.tile([1, NCH * B], F32)
        nc.scalar.dma_start(out=selrow[:], in_=selr_i[:])
        selbig = singles.tile([B, NCH * P], F32)
        nc.scalar.dma_start(out=selbig[:], in_=selb_i[:])

        # ---------------- pass 1: per-chunk column sums ----------------
        acc_ps = psumS.tile([B, 2 * C], F32)
        for ci in range(NCH):
            xa = poolA.tile([P, WC * C], BF16)
            nc.sync.dma_start(out=xa[:],
                              in_=xbf_i[:, ci * WC * C:(ci + 1) * WC * C])
            xs = poolS.tile([P, WC * C], BF16)
            nc.scalar.activation(out=xs[:], in_=xa[:],
                                 func=mybir.ActivationFunctionType.Square,
                                 scale=1.0)
            xa_r = xa[:].rearrange("p (w c) -> p w c", c=C)
            xs_r = xs[:].rearrange("p (w c) -> p w c", c=C)

            ps12 = psumA.tile([1, 2 * C], F32)
            for w in range(WC):
                nc.tensor.matmul(out=ps12[:, 0:C], lhsT=ones128[:],
                                 rhs=xa_r[:, w, :],
                                 start=(w == 0), stop=(w == WC - 1))
                nc.tensor.matmul(out=ps12[:, C:2 * C], lhsT=ones128[:],
                                 rhs=xs_r[:, w, :],
                                 start=(w == 0), stop=(w == WC - 1))
            cp12 = poolCP.tile([1, 2 * C], F32)
            nc.vector.tensor_copy(out=cp12[:], in_=ps12[:])
            # scatter chunk sum into this chunk's segment row
            nc.tensor.matmul(out=acc_ps[:],
                             lhsT=selrow[:, ci * B:(ci + 1) * B],
                             rhs=cp12[:],
                             start=(ci == 0), stop=(ci == NCH - 1))

        acc_sb = singles.tile([B, 2 * C], F32)
        nc.vector.tensor_copy(out=acc_sb[:], in_=acc_ps[:])

        # ---------------- all-reduce partials ----------------
        nc.scalar.dma_start(out=cc_in[:], in_=acc_sb[:])
        nc.gpsimd.collective_compute(
            "AllReduce", mybir.AluOpType.add,
            replica_groups=[list(range(NCORES))],
            ins=[cc_in[:]], outs=[cc_out[:]])
        s12 = singles.tile([B, 2 * C], F32)
        nc.scalar.dma_start(out=s12[:], in_=cc_out[:])

        # ---------------- stats -> A/D tables ----------------
        hs = singles.tile([B, 8], F32)
        nc.scalar.dma_start(out=hs[:], in_=hs_i[:])
        c82 = singles.tile([B, 2], F32)
        nc.scalar.dma_start(out=c82[:], in_=c82_i[:])
        w8 = singles.tile([B, C], F32)
        nc.scalar.dma_start(out=w8[:], in_=w_i[:].to_broadcast([B, C]))
        b8 = singles.tile([B, C], F32)
        nc.scalar.dma_start(out=b8[:], in_=b_i[:].to_broadcast([B, C]))

        S1g = s12[:, 0:C]
        S2g = s12[:, C:2 * C]
        invc = hs[:, 0:1]

        mean_in = singles.tile([B, C], F32)
        nc.vector.tensor_scalar(out=mean_in[:], in0=S1g, scalar1=invc,
                                scalar2=None, op0=mybir.AluOpType.mult)
        E2 = singles.tile([B, C], F32)
        nc.vector.tensor_scalar(out=E2[:], in0=S2g, scalar1=invc,
                                scalar2=None, op0=mybir.AluOpType.mult)
        var_in = singles.tile([B, C], F32)
        nc.vector.tensor_tensor(out=var_in[:], in0=mean_in[:], in1=mean_in[:],
                                op=mybir.AluOpType.mult)
        nc.vector.tensor_tensor(out=var_in[:], in0=E2[:], in1=var_in[:],
                                op=mybir.AluOpType.subtract)

        mean_ln = singles.tile([B, 1], F32)
        nc.vector.reduce_sum(out=mean_ln[:], in_=mean_in[:],
                             axis=mybir.AxisListType.X)
        nc.vector.tensor_scalar(out=mean_ln[:], in0=mean_ln[:],
                                scalar1=1.0 / C, scalar2=None,
                                op0=mybir.AluOpType.mult)
        E2_ln = singles.tile([B, 1], F32)
        nc.vector.reduce_sum(out=E2_ln[:], in_=E2[:],
                             axis=mybir.AxisListType.X)
        var_ln = singles.tile([B, 1], F32)
        nc.vector.tensor_scalar(out=E2_ln[:], in0=E2_ln[:], scalar1=1.0 / C,
                                scalar2=None, op0=mybir.AluOpType.mult)
        nc.vector.tensor_tensor(out=var_ln[:], in0=mean_ln[:], in1=mean_ln[:],
                                op=mybir.AluOpType.mult)
        nc.vector.tensor_tensor(out=var_ln[:], in0=E2_ln[:], in1=var_ln[:],
                                op=mybir.AluOpType.subtract)

        # column sums over segments (M=1 matmuls, results on partition 0)
        ps_cs = psumS.tile([1, 4 * C], F32)
        nc.tensor.matmul(out=ps_cs[:, 0:2 * C], lhsT=c82[:, 0:1], rhs=s12[:],
                         start=True, stop=True)
        nc.tensor.matmul(out=ps_cs[:, 2 * C:4 * C], lhsT=c82[:, 1:2],
                         rhs=s12[:], start=True, stop=True)
        cs1 = singles.tile([1, 2 * C], F32)
        nc.vector.tensor_copy(out=cs1[:], in_=ps_cs[:, 0:2 * C])
        cs2 = singles.tile([1, 2 * C], F32)
        nc.vector.tensor_copy(out=cs2[:], in_=ps_cs[:, 2 * C:4 * C])
        # mean_bn = cs1[0, 0:C] ;  S2/(N-1) = cs2[0, C:2C]
        mvbn = singles.tile([1, 2 * C], F32)
        nc.vector.tensor_copy(out=mvbn[:, 0:C], in_=cs1[:, 0:C])
        mbn2 = singles.tile([1, C], F32)
        nc.vector.tensor_tensor(out=mbn2[:], in0=cs1[:, 0:C],
                                in1=cs1[:, 0:C], op=mybir.AluOpType.mult)
        nc.vector.tensor_scalar(out=mbn2[:], in0=mbn2[:],
                                scalar1=float(NTOT) / float(NTOT - 1),
                                scalar2=None, op0=mybir.AluOpType.mult)
        nc.vector.tensor_tensor(out=mvbn[:, C:2 * C], in0=cs2[:, C:2 * C],
                                in1=mbn2[:], op=mybir.AluOpType.subtract)

        # broadcast [1,128] -> [8,128] via K=1 matmul with ones
        ones18 = singles.tile([1, B], F32)
        nc.vector.memset(ones18[:], 1.0)
        ps_bc = psumS.tile([B, 2 * C], F32)
        nc.tensor.matmul(out=ps_bc[:], lhsT=ones18[:], rhs=mvbn[:],
                         start=True, stop=True)
        bc = singles.tile([B, 2 * C], F32)
        nc.vector.tensor_copy(out=bc[:], in_=ps_bc[:])

        # mean = mw0*mean_in + mw1*mean_ln + mw2*mean_bn
        mls = singles.tile([B, 1], F32)
        nc.vector.tensor_tensor(out=mls[:], in0=mean_ln[:], in1=hs[:, 2:3],
                                op=mybir.AluOpType.mult)
        mean = singles.tile([B, C], F32)
        nc.vector.tensor_scalar(out=mean[:], in0=mean_in[:],
                                scalar1=hs[:, 1:2], scalar2=mls[:],
                                op0=mybir.AluOpType.mult,
                                op1=mybir.AluOpType.add)
        t2 = singles.tile([B, C], F32)
        nc.vector.tensor_scalar(out=t2[:], in0=bc[:, 0:C], scalar1=hs[:, 3:4],
                                scalar2=None, op0=mybir.AluOpType.mult)
        nc.vector.tensor_tensor(out=mean[:], in0=mean[:], in1=t2[:],
                                op=mybir.AluOpType.add)

        # var = vw0*var_in + vw1*var_ln + vw2*var_bn
        vls = singles.tile([B, 1], F32)
        nc.vector.tensor_tensor(out=vls[:], in0=var_ln[:], in1=hs[:, 5:6],
                                op=mybir.AluOpType.mult)
        var = singles.tile([B, C], F32)
        nc.vector.tensor_scalar(out=var[:], in0=var_in[:],
                                scalar1=hs[:, 4:5], scalar2=vls[:],
                                op0=mybir.AluOpType.mult,
                                op1=mybir.AluOpType.add)
        nc.vector.tensor_scalar(out=t2[:], in0=bc[:, C:2 * C],
                                scalar1=hs[:, 6:7], scalar2=None,
                                op0=mybir.AluOpType.mult)
        nc.vector.tensor_tensor(out=var[:], in0=var[:], in1=t2[:],
                                op=mybir.AluOpType.add)

        # inv_std = 1/sqrt(var+eps);  A = inv_std*w ; D = b - mean*A
        istd = singles.tile([B, C], F32)
        nc.scalar.activation(out=istd[:], in_=var[:],
                             func=mybir.ActivationFunctionType.Sqrt,
                             bias=hs[:, 7:8], scale=1.0)
        nc.vector.reciprocal(out=istd[:], in_=istd[:])
        AD = singles.tile([B, 2 * C], F32)
        nc.vector.tensor_tensor(out=AD[:, 0:C], in0=istd[:], in1=w8[:],
                                op=mybir.AluOpType.mult)
        mA = singles.tile([B, C], F32)
        nc.vector.tensor_tensor(out=mA[:], in0=mean[:], in1=AD[:, 0:C],
                                op=mybir.AluOpType.mult)
        nc.vector.tensor_tensor(out=AD[:, C:2 * C], in0=b8[:], in1=mA[:],
                                op=mybir.AluOpType.subtract)

        # ---------------- pass 2: normalize ----------------
        for ci in range(NCH):
            xf = poolC.tile([P, WC * C], F32)
            nc.sync.dma_start(out=xf[:],
                              in_=xf_i[:, ci * WC * C:(ci + 1) * WC * C])
            xf_r = xf[:].rearrange("p (w c) -> p w c", c=C)

            ps_ad = psumB.tile([P, 2 * C], F32)
            nc.tensor.matmul(out=ps_ad[:],
                             lhsT=selbig[:, ci * P:(ci + 1) * P],
                             rhs=AD[:], start=True, stop=True)
            ad128 = poolAD.tile([P, 2 * C], F32)
            nc.vector.tensor_copy(out=ad128[:], in_=ps_ad[:])
            a_b = ad128[:, 0:C]
            d_b = ad128[:, C:2 * C]
            a_bc = bass.AP(tensor=a_b.tensor, offset=a_b.offset,
                           ap=[a_b.ap[0], [0, WC], a_b.ap[1]])
            d_bc = bass.AP(tensor=d_b.tensor, offset=d_b.offset,
                           ap=[d_b.ap[0], [0, WC], d_b.ap[1]])

            outc = poolO.tile([P, WC * C], F32)
            outc_r = outc[:].rearrange("p (w c) -> p w c", c=C)
            tmp = poolT.tile([P, WC * C], F32)
            tmp_r = tmp[:].rearrange("p (w c) -> p w c", c=C)
            nc.vector.tensor_tensor(out=tmp_r, in0=xf_r, in1=a_bc,
                                    op=mybir.AluOpType.mult)
            nc.vector.tensor_tensor(out=outc_r, in0=tmp_r, in1=d_bc,
                                    op=mybir.AluOpType.add)
            nc.gpsimd.dma_start(out=out_o[:, ci * WC * C:(ci + 1) * WC * C],
                                in_=outc[:])

    nc.compile()
    return nc


def _get_nc():
    if "nc" not in _CACHE:
        _CACHE["nc"] = _build()
    return _CACHE["nc"]


def _softmax32(v):
    v = np.asarray(v, np.float32)
    e = np.exp(v - v.max())
    return (e / e.sum()).astype(np.float32)


def _prep_inputs(x, batch_ids, weight, bias, mean_weight, var_weight):
    x = np.asarray(x, np.float32)
    ids = np.asarray(batch_ids, np.int32)

    counts = np.bincount(ids, minlength=B)
    counts_c = np.maximum(counts, 1)
    mw = _softmax32(mean_weight)
    vw = _softmax32(var_weight)

    hs = np.zeros((B, 8), np.float32)
    hs[:, 0] = (1.0 / counts_c.astype(np.float64)).astype(np.float32)
    hs[:, 1] = mw[0]
    hs[:, 2] = mw[1]
    hs[:, 3] = mw[2]
    hs[:, 4] = vw[0]
    hs[:, 5] = vw[1]
    hs[:, 6] = vw[2]
    hs[:, 7] = EPS
    c82 = np.zeros((B, 2), np.float32)
    c82[:, 0] = 1.0 / NTOT
    c82[:, 1] = 1.0 / (NTOT - 1)
    wt = np.ascontiguousarray(np.asarray(weight, np.float32).reshape(1, C))
    bs = np.ascontiguousarray(np.asarray(bias, np.float32).reshape(1, C))

    # --- sort points by segment; each 8192-point chunk single-segment ---
    order = np.argsort(ids, kind="stable")        # sorted point order
    nchunks_b = (counts + CHP - 1) // CHP          # chunks per segment
    assert nchunks_b.sum() <= TOTCH, "segment sizes exceed chunk capacity"
    chunk_seg = np.full(TOTCH, -1, np.int64)
    seg_chunk_start = np.zeros(B + 1, np.int64)
    pos = 0
    for b in range(B):
        chunk_seg[pos:pos + nchunks_b[b]] = b
        seg_chunk_start[b] = pos
        pos += nchunks_b[b]
    seg_chunk_start[B] = pos

    # device slot for each sorted point
    cum = np.zeros(B + 1, np.int64)
    cum[1:] = np.cumsum(counts)
    ids_sorted = ids[order]
    within = np.arange(NTOT, dtype=np.int64) - cum[ids_sorted]
    dev_slot = seg_chunk_start[ids_sorted] * CHP + within

    xdev = np.zeros((NCORES * NLP, C), np.float32)
    xdev[dev_slot] = x[order]

    # chunk one-hot selectors
    selr_all = np.zeros((TOTCH, B), np.float32)
    valid = chunk_seg >= 0
    selr_all[np.arange(TOTCH)[valid], chunk_seg[valid]] = 1.0

    in_maps = []
    for i in range(NCORES):
        xp = xdev[i * NLP:(i + 1) * NLP]
        xB = np.ascontiguousarray(
            xp.reshape(W, P, C).transpose(1, 0, 2)).reshape(P, W * C)
        selr = np.ascontiguousarray(
            selr_all[i * NCH:(i + 1) * NCH].reshape(1, NCH * B))
        # selb[b, ci*128+p] = (chunk ci of this core has segment b)
        selb = np.ascontiguousarray(
            np.repeat(selr_all[i * NCH:(i + 1) * NCH].T[:, :, None],
                      P, axis=2).reshape(B, NCH * P))
        in_maps.append(dict(
            xbf=np.ascontiguousarray(xB.astype(ml_dtypes.bfloat16)),
            xf=xB, selr=selr, selb=selb,
            wt=wt, bs=bs, hs=hs, c82=c82))
    _CACHE["scatter"] = (order, dev_slot)
    return in_maps


def _postprocess(res):
    order, dev_slot = _CACHE["scatter"]
    flat = np.empty((NCORES * NLP, C), np.float32)
    for i in range(NCORES):
        o = np.asarray(res.results[i]["out"], np.float32)
        flat[i * NLP:(i + 1) * NLP] = o.reshape(
            P, W, C).transpose(1, 0, 2).reshape(NLP, C)
    out = np.empty((NTOT, C), np.float32)
    out[order] = flat[dev_slot]
    return out


def kernel(x, batch_ids, weight, bias, mean_weight, var_weight):
    nc = _get_nc()
    in_maps = _prep_inputs(x, batch_ids, weight, bias,
                           mean_weight, var_weight)
    res = run_bass_kernel_spmd(nc, in_maps, list(range(NCORES)))
    _CACHE["last_result"] = res
    return _postprocess(res)
